# revision 1
# baseline (speedup 1.0000x reference)
"""Trainium2 Bass kernel for the Boltzmann-machine recurrence, v3.

v3 on top of the data-parallel bf16 design:
  - clamp_x makes the x-block contribution to every step's matmul constant:
    C = xT.T @ mwT[0:1024, :].  Step 0 computes C in PSUM (it IS step 0's
    pre-relu output), caches it in SBUF as two bf16 planes (hi + residual
    for fp32-grade accuracy), and later steps re-inject it with two
    identity matmuls per bank instead of 8 k-chunk matmuls (PE 150 vs 192
    matmuls per step).
  - 16 of the 24 dynamic weight k-chunks stay resident in SBUF (~96 KB of
    224 KB per partition); only 6 MB/step streams from HBM, hidden under
    the PE work.
  - hid banks are accumulated and drained before y banks, so the norm
    chain (Square+sqrt+recip+scale+transpose) of hid overlaps the y-bank
    matmuls.
"""

import numpy as np
import ml_dtypes
from contextlib import ExitStack

import concourse.bass as bass
from concourse import bacc
import concourse.mybir as mybir
import concourse.tile as tile
from concourse.bass_utils import run_bass_kernel_spmd
from concourse.masks import make_identity

IN, OUT, HID = 1024, 1024, 2048
L = IN + OUT + HID              # 4096
B = 1024
N_CORES = 8
BC = B // N_CORES               # 128 batch rows per core
NK = L // 128                   # 32 contraction chunks
ND = NK - 8                     # 24 dynamic chunks (global k = 8..31)
RES = 14                        # dynamic chunks resident in SBUF (kl < RES)
JW = L - IN                     # 3072 computed output columns
YW, HW = OUT, HID               # local col split: y = [0,1024), hid = [1024,3072)
EPS = 1e-12

F32 = mybir.dt.float32
BF16 = mybir.dt.bfloat16
MMDT = BF16

_COMPILED = {}


def _build(n_steps: int):
    nc = bacc.Bacc(None, target_bir_lowering=False)
    wx_ext = nc.declare_dram_parameter("wx", [8, 128, JW], MMDT, isOutput=False)
    wh_ext = nc.declare_dram_parameter("wh", [ND, 128, HW], MMDT, isOutput=False)
    wy_ext = nc.declare_dram_parameter("wy", [ND, 128, YW], MMDT, isOutput=False)
    xt_ext = nc.declare_dram_parameter("xt", [128, IN], MMDT, isOutput=False)
    out_ext = nc.declare_dram_parameter("out", [BC, JW], F32, isOutput=True)

    with ExitStack() as ctx:
        tc = ctx.enter_context(tile.TileContext(nc))
        const_pool = ctx.enter_context(tc.tile_pool(name="const", bufs=1))
        actt_pool = ctx.enter_context(tc.tile_pool(name="actt", bufs=1))
        wres_pool = ctx.enter_context(tc.tile_pool(name="wres", bufs=1))
        wx_pool = ctx.enter_context(tc.tile_pool(name="wx", bufs=2))
        wh_pool = ctx.enter_context(tc.tile_pool(name="wh", bufs=9))
        wy_pool = ctx.enter_context(tc.tile_pool(name="wy", bufs=9))
        psum_pool = ctx.enter_context(tc.tile_pool(name="psum", bufs=1, space="PSUM"))
        tpsum_pool = ctx.enter_context(tc.tile_pool(name="tpsum", bufs=2, space="PSUM"))
        stage_pool = ctx.enter_context(tc.tile_pool(name="stage", bufs=2))
        norm_pool = ctx.enter_context(tc.tile_pool(name="norm", bufs=2))
        fin_pool = ctx.enter_context(tc.tile_pool(name="fin", bufs=1))

        ident = const_pool.tile([128, 128], MMDT)
        make_identity(nc, ident)

        # actt[p, k*128 + b] = act[b, k*128 + p]; chunks 0..7 = xT (constant)
        actt = actt_pool.tile([128, NK * 128], MMDT)
        nc.sync.dma_start(actt[:, 0:IN], xt_ext[:])

        # resident dynamic weights: kl in [8, 24) -- the actt hid chunks,
        # which each step's passes contract FIRST (they are transposed early
        # in the previous step); the streamed kl in [0, 8) come last and get
        # a full step of prefetch slack.
        wres_h = wres_pool.tile([128, RES * HW], MMDT)
        wres_y = wres_pool.tile([128, RES * YW], MMDT)
        KL_ORDER = list(range(8, 24)) + list(range(8))

        # C planes: constant x-block contribution, cols local [0, JW)
        c_hi = const_pool.tile([128, JW], MMDT)
        c_lo = const_pool.tile([128, JW], MMDT)

        HB = [2, 3, 4, 5]       # psum banks for hid (local cols 1024..3072)
        YB = [0, 1]             # psum banks for y   (local cols 0..1024)

        def bank_cols(jb):
            return jb * 512, (jb + 1) * 512

        for s in range(n_steps):
            last = s == n_steps - 1
            psums = {}

            if s == 0:
                # x chunks only; also extract C planes from psum
                for jb in range(6):
                    psums[jb] = psum_pool.tile([128, 512], F32,
                                               name=f"ps{jb}", tag=f"ps{jb}")
                for k in range(8):
                    wxt = wx_pool.tile([128, JW], MMDT, name="wxt", tag="wxt")
                    nc.sync.dma_start(wxt[:], wx_ext[k])
                    for jb in range(6):
                        lo, hi = bank_cols(jb)
                        nc.tensor.matmul(
                            psums[jb][:], lhsT=actt[:, k * 128:(k + 1) * 128],
                            rhs=wxt[:, lo:hi], start=(k == 0), stop=(k == 7))
                for jb in range(6):
                    lo, hi = bank_cols(jb)
                    nc.scalar.copy(c_hi[:, lo:hi], psums[jb][:])
                    nc.vector.tensor_sub(c_lo[:, lo:hi], psums[jb][:],
                                         c_hi[:, lo:hi])
                # resident weights load behind step 0's own DMAs
                for kl in range(8, 8 + RES):
                    nc.sync.dma_start(wres_h[:, (kl - 8) * HW:(kl - 7) * HW],
                                      wh_ext[kl])
                    nc.sync.dma_start(wres_y[:, (kl - 8) * YW:(kl - 7) * YW],
                                      wy_ext[kl])
            else:
                # step 1 contracts its streamed chunks first (the resident
                # load may still be in flight); later steps contract resident
                # hid chunks first (freshest transposes last)
                order = ([kl for kl in KL_ORDER if not 8 <= kl < 8 + RES]
                         + [kl for kl in KL_ORDER if 8 <= kl < 8 + RES]
                         ) if s == 1 else KL_ORDER
                # pass H then pass Y, C re-injected via identity matmuls
                for group, wres, wext, wpool, wwid, colbase in (
                    (HB, wres_h, wh_ext, wh_pool, HW, YW),
                    (YB, wres_y, wy_ext, wy_pool, YW, 0),
                ):
                    # hoist this pass's stream DMAs for maximum prefetch lead
                    wts = {}
                    for kl in order:
                        if not 8 <= kl < 8 + RES:
                            wt = wpool.tile([128, wwid], MMDT,
                                            name="wst", tag=f"wst{wwid}")
                            nc.sync.dma_start(wt[:], wext[kl])
                            wts[kl] = wt
                    for jb in group:
                        psums[jb] = psum_pool.tile([128, 512], F32,
                                                   name=f"ps{jb}", tag=f"ps{jb}")
                        lo, hi = bank_cols(jb)
                        nc.tensor.matmul(psums[jb][:], lhsT=ident[:],
                                         rhs=c_hi[:, lo:hi],
                                         start=True, stop=False)
                        nc.tensor.matmul(psums[jb][:], lhsT=ident[:],
                                         rhs=c_lo[:, lo:hi],
                                         start=False, stop=False)
                    for ki, kl in enumerate(order):
                        if 8 <= kl < 8 + RES:
                            rhs_all = wres[:, (kl - 8) * wwid:(kl - 7) * wwid]
                        else:
                            rhs_all = wts[kl][:]
                        kg = 8 + kl
                        for bi, jb in enumerate(group):
                            nc.tensor.matmul(
                                psums[jb][:],
                                lhsT=actt[:, kg * 128:(kg + 1) * 128],
                                rhs=rhs_all[:, bi * 512:(bi + 1) * 512],
                                start=False, stop=(ki == ND - 1))

            if not last:
                act_sb = stage_pool.tile([128, JW], MMDT, tag="act_sb", bufs=1)
                # hid first: drain, norm, scale, transpose
                for jb in HB:
                    lo, hi = bank_cols(jb)
                    nc.scalar.activation(act_sb[:, lo:hi], psums[jb][:],
                                         mybir.ActivationFunctionType.Relu)
                hid = act_sb[:, YW:JW]
                sq = stage_pool.tile([128, HID], MMDT, tag="sq", bufs=1)
                ssq = norm_pool.tile([128, 1], F32, tag="ssq")
                nc.scalar.activation(sq[:], hid,
                                     mybir.ActivationFunctionType.Square,
                                     accum_out=ssq[:])
                nrm = norm_pool.tile([128, 1], F32, tag="nrm")
                nc.scalar.sqrt(nrm[:], ssq[:])
                nc.vector.tensor_scalar_max(nrm[:], nrm[:], EPS)
                rinv = norm_pool.tile([128, 1], F32, tag="rinv")
                nc.vector.reciprocal(rinv[:], nrm[:])
                hid_n = stage_pool.tile([128, HID], MMDT, tag="hid_n", bufs=1)
                nc.vector.tensor_scalar_mul(hid_n[:], hid, rinv[:])

                # transposes: hid chunks -> actt chunks 16..31 (2 groups of 8)
                for g in range(2):
                    pt = tpsum_pool.tile([128, 1024], MMDT, name="pt", tag="pt")
                    for u in range(8):
                        c = g * 8 + u
                        nc.tensor.transpose(pt[:, u * 128:(u + 1) * 128],
                                            hid_n[:, c * 128:(c + 1) * 128],
                                            ident[:])
                    nc.vector.tensor_copy(
                        actt[:, (16 + g * 8) * 128:(24 + g * 8) * 128], pt[:])

                # y: drain + transpose -> actt chunks 8..15
                for jb in YB:
                    lo, hi = bank_cols(jb)
                    nc.scalar.activation(act_sb[:, lo:hi], psums[jb][:],
                                         mybir.ActivationFunctionType.Relu)
                pt = tpsum_pool.tile([128, 1024], MMDT, name="pt", tag="pt")
                for u in range(8):
                    nc.tensor.transpose(pt[:, u * 128:(u + 1) * 128],
                                        act_sb[:, u * 128:(u + 1) * 128],
                                        ident[:])
                nc.vector.tensor_copy(actt[:, 8 * 128:16 * 128], pt[:])
            else:
                out_sb = fin_pool.tile([128, JW], F32, tag="out_sb")
                for jb in HB + YB:
                    lo, hi = bank_cols(jb)
                    nc.scalar.activation(out_sb[:, lo:hi], psums[jb][:],
                                         mybir.ActivationFunctionType.Relu)
                hid = out_sb[:, YW:JW]
                hid_n = fin_pool.tile([128, HID], F32, tag="hid_nf")
                ssq = norm_pool.tile([128, 1], F32, tag="ssq")
                nc.scalar.activation(hid_n[:], hid,
                                     mybir.ActivationFunctionType.Square,
                                     accum_out=ssq[:])
                nrm = norm_pool.tile([128, 1], F32, tag="nrm")
                nc.scalar.sqrt(nrm[:], ssq[:])
                nc.vector.tensor_scalar_max(nrm[:], nrm[:], EPS)
                rinv = norm_pool.tile([128, 1], F32, tag="rinv")
                nc.vector.reciprocal(rinv[:], nrm[:])
                nc.vector.tensor_scalar_mul(hid_n[:], hid, rinv[:])

                nc.sync.dma_start(out_ext[:, 0:YW], out_sb[:, 0:YW])
                nc.sync.dma_start(out_ext[:, YW:JW], hid_n[:])
    nc.finalize()
    return nc


def _prepack(x, W, A):
    bf = ml_dtypes.bfloat16
    mw = W.astype(np.float32) * A.astype(np.float32).T
    mwT = np.ascontiguousarray(mw.T[:, IN:])                 # [L, JW]
    mwT_bf = mwT.astype(bf)
    wx = np.ascontiguousarray(mwT_bf[:IN].reshape(8, 128, JW))
    dyn = mwT_bf[IN:].reshape(ND, 128, JW)
    wy = np.ascontiguousarray(dyn[:, :, :YW])
    wh = np.ascontiguousarray(dyn[:, :, YW:])

    xts = []
    for c in range(N_CORES):
        xc = x[c * BC:(c + 1) * BC]
        xt = xc.T.reshape(IN // 128, 128, BC).transpose(1, 0, 2).reshape(128, IN)
        xts.append(np.ascontiguousarray(xt.astype(bf)))
    return wx, wh, wy, xts


def run(x, y, W, A, n, trace=False):
    n = int(n)
    x = np.asarray(x, dtype=np.float32)
    assert x.shape == (B, IN)

    if n == 0:
        return np.concatenate(
            [x, np.zeros((B, OUT), np.float32), np.zeros((B, HID), np.float32)],
            axis=1), None

    wx, wh, wy, xts = _prepack(x, np.asarray(W), np.asarray(A))

    if n not in _COMPILED:
        _COMPILED[n] = _build(n)
    nc = _COMPILED[n]

    in_maps = [{"wx": wx, "wh": wh, "wy": wy, "xt": xts[c]}
               for c in range(N_CORES)]
    res = run_bass_kernel_spmd(nc, in_maps, list(range(N_CORES)), trace=trace)
    parts = [res.results[c]["out"] for c in range(N_CORES)]
    right = np.concatenate(parts, axis=0)
    return np.concatenate([x, right.astype(np.float32)], axis=1), res


def kernel(x, y, W, A, n):
    out, _ = run(x, y, W, A, n)
    return out

